# revision 11
# baseline (speedup 1.0000x reference)
"""Multi-head attention (B=4, T=2048, D=1024, H=16) on 8 Trainium2 cores.

Sharding: batch (4-way) x head-half (2-way) -> 8 cores.
Core c handles batch b = c//2 and heads g*8..g*8+8 where g = c%2.

v3: row-tiled concurrent score matmuls.
  - Scores for a HEAD PAIR run as two concurrent K=64 matmuls in the two
    64-row halves of the PE array (tile_position row tiling, auto-derived
    from the AP base partitions). qk_sb already stores each j-tile as
    [128 = headpair hd, T], so head 2j lives in partitions 0-63 and head
    2j+1 in 64-127 for both q and kT: no zero-padded qpad staging needed.
  - Loop iterates (pair j, query-chunk c of 512, k-tile). Each iteration:
    2 concurrent score matmuls -> one [128,1024] exp (chunk for A || B)
    -> 2 AV matmuls (one per head, ones-column augmented v for the
    denominators). PSUM: scores 2x2 banks (double buffer), AV 2 banks,
    filler 2 banks = 8.
  - Softmax denominator row copies moved off the scalar engine (DVE) so
    ACT runs pure Exp back-to-back; normalization per (pair, chunk) is
    one reciprocal + DRAM-bounce broadcast + one [128,512] DVE multiply
    for both heads at once. Last two units broadcast 1/d through the PE
    instead (rank-1 fp32 matmuls) to skip the bounce latency on the tail.
  - q/k projections run as fp8e4 DoubleRow matmuls (2 k-tiles per
    instruction). Inputs are scaled (x*16, W*64) into the fp8 normal
    range; the 1/1024 fixup is folded into the psum->SBUF evacuation.
    v stays bf16. Biases fold into the psum evacuations.
  - Filler GEMMs (remaining projections + out-projection) weave into the
    attention stream per 16-iteration windows with due-date slopes.
  - Output is bf16 (halves DMA); host sums the two head-half partials.

Host: transposes/reshapes inputs per core (bf16/fp8), sums partials,
adds out_b.
"""

import numpy as np
import ml_dtypes
from contextlib import ExitStack

import concourse.bass as bass
import concourse.tile as tile
from concourse import bacc, mybir
from concourse.bass_utils import run_bass_kernel_spmd

BF16_NP = ml_dtypes.bfloat16
FP8_NP = ml_dtypes.float8_e4m3

B, T, D = 4, 2048, 1024
H, HD = 16, 64
P = 128
NC = 8
HPC = 8          # heads per core
JC = HPC * HD    # 512 head-dim columns per core
KT = D // P      # 8 contraction tiles for QKV
TT = T // P      # 16 t tiles
TCH = T // 512   # 4 t chunks of 512
NPAIR = HPC // 2  # 4 head pairs per core
F32 = mybir.dt.float32
BF16 = mybir.dt.bfloat16
FP8 = mybir.dt.float8e4

USE_FP8_KQ = True
X8_SCALE = 16.0
W8_SCALE = 64.0
KQ_FIX = 1.0 / (X8_SCALE * W8_SCALE)

_cached = {}


def build_program():
    nc = bacc.Bacc("TRN2", target_bir_lowering=False, debug=False,
                   enable_asserts=True, num_devices=NC)

    xt16_d = nc.dram_tensor("xt16", [TCH, P, KT, 512], BF16,
                            kind="ExternalInput").ap()
    if USE_FP8_KQ:
        xt8_d = nc.dram_tensor("xt8", [TCH, P, KT // 2, 2, 512], FP8,
                               kind="ExternalInput").ap()
        wqk_d = nc.dram_tensor("wqk", [P, 8, KT // 2, 2, P], FP8,
                               kind="ExternalInput").ap()
    else:
        wqk_d = nc.dram_tensor("wqk", [P, KT, 2 * JC], BF16,
                               kind="ExternalInput").ap()
    wv_d = nc.dram_tensor("wv", [P, KT, JC], BF16, kind="ExternalInput").ap()
    wo_d = nc.dram_tensor("wo", [P, JC // P, D], BF16,
                          kind="ExternalInput").ap()
    bqk_d = nc.dram_tensor("bqk", [P, 8], F32, kind="ExternalInput").ap()
    ident_d = nc.dram_tensor("ident", [P, P], BF16, kind="ExternalInput").ap()
    bvb_d = nc.dram_tensor("bvb", [P, JC], BF16, kind="ExternalInput").ap()
    out_d = nc.dram_tensor("out", [T, D], BF16, kind="ExternalOutput").ap()

    EXP = mybir.ActivationFunctionType.Exp
    COPY = mybir.ActivationFunctionType.Copy
    DR = mybir.MatmulPerfMode.DoubleRow
    SC = 0.125  # 1/sqrt(HD)

    with tile.TileContext(nc) as tc:
        with ExitStack() as ctx:
            persist = ctx.enter_context(tc.tile_pool(name="persist", bufs=1))
            xt16_sb = persist.tile([P, TCH, KT, 512], BF16, tag="xt16")
            if USE_FP8_KQ:
                xt8_sb = persist.tile([P, TCH, KT // 2, 2, 512], FP8,
                                      tag="xt8")
                wqk_sb = persist.tile([P, 8, KT // 2, 2, P], FP8, tag="wqk")
            else:
                wqk_sb = persist.tile([P, KT, 2 * JC], BF16, tag="wqk")
            wv_sb = persist.tile([P, KT, JC], BF16, tag="wv")
            wo_sb = persist.tile([P, JC // P, D], BF16, tag="wo")
            bqk_sb = persist.tile([P, 8], F32, tag="bqk")
            ident_sb = persist.tile([P, P], BF16, tag="ident")
            bvb_sb = persist.tile([P, JC], BF16, tag="bvb")
            qk_sb = persist.tile([P, 8, T], BF16, tag="qk")
            # [t, 8 x v(64)]: pure v; denominators come from ones-matmuls
            vaug_f = persist.tile([P, TT, JC], BF16, tag="vaug")
            ot_sb = persist.tile([P, JC // P, T], BF16, tag="ot")
            # out-projection partial sums over head-pairs 0..2 (bf16), so
            # most of the out-proj runs in the late windows
            ost_part = persist.tile([P, TT, D], BF16, tag="ostp")

            # ---- input DMAs (program order = queue order) ----
            if USE_FP8_KQ:
                for tci in range(2):
                    nc.sync.dma_start(xt8_sb[:, tci], xt8_d[tci])
                nc.sync.dma_start(xt16_sb[:, 0], xt16_d[0])
                for tci in range(2, TCH):
                    nc.sync.dma_start(xt8_sb[:, tci], xt8_d[tci])
                for tci in range(1, TCH):
                    nc.sync.dma_start(xt16_sb[:, tci], xt16_d[tci])
                nc.gpsimd.dma_start(wqk_sb[:], wqk_d[:])
            else:
                for tci in range(TCH):
                    nc.sync.dma_start(xt16_sb[:, tci], xt16_d[tci])
                nc.gpsimd.dma_start(wqk_sb[:], wqk_d[:])
            nc.gpsimd.dma_start(wv_sb[:], wv_d[:])
            nc.gpsimd.dma_start(bqk_sb[:], bqk_d[:])
            nc.gpsimd.dma_start(ident_sb[:], ident_d[:])
            nc.gpsimd.dma_start(bvb_sb[:], bvb_d[:])
            nc.gpsimd.dma_start(wo_sb[:], wo_d[:])

            ones1 = persist.tile([1, HD], F32, tag="ones1")
            nc.gpsimd.memset(ones1[:], 1.0)
            ones_av = persist.tile([P, 1], BF16, tag="ones_av")
            nc.gpsimd.memset(ones_av[:], 1.0)
            vaug = vaug_f.rearrange(
                "p t (h e) -> p t h e", h=HPC)          # [128, 16, 8, 64]

            AV_LAG = 4  # AV trails exp by this many extra iterations
            wtpool = ctx.enter_context(
                tc.tile_pool(name="wtpool", bufs=AV_LAG + 3))
            ddpool = ctx.enter_context(tc.tile_pool(name="ddpool", bufs=1))
            rcpool = ctx.enter_context(tc.tile_pool(name="rcpool", bufs=2))
            rbpool = ctx.enter_context(tc.tile_pool(name="rbpool", bufs=2))
            ostpool = ctx.enter_context(tc.tile_pool(name="ostpool", bufs=2))
            rdpool = ctx.enter_context(
                tc.tile_pool(name="rdpool", bufs=2, space="DRAM"))
            pss = ctx.enter_context(
                tc.tile_pool(name="pss", bufs=2, space="PSUM"))
            avp = ctx.enter_context(
                tc.tile_pool(name="avp", bufs=1, space="PSUM"))
            psf = ctx.enter_context(
                tc.tile_pool(name="psf", bufs=2, space="PSUM"))

            # ---------------- filler group builders ----------------
            # Each group is a list of closures; each closure emits one PE
            # matmul (the last also emits the psum evacuation on DVE).

            def kq_group(jcol, tci):
                """qk_sb j-tile jcol (0-3 = q j, 4-7 = kT j) over t-chunk."""
                wcol = jcol * P if jcol < 4 else JC + (jcol - 4) * P
                tsl = slice(tci * 512, (tci + 1) * 512)
                steps = []
                box = {}
                nsteps = KT // 2 if USE_FP8_KQ else KT

                def mk(i):
                    first, last = i == 0, i == nsteps - 1

                    def step():
                        if first:
                            box["ps"] = psf.tile([P, 512], F32, tag="psf",
                                                 name=f"kq_{jcol}_{tci}")
                        if USE_FP8_KQ:
                            nc.tensor.matmul(
                                box["ps"][:],
                                wqk_sb[:, jcol, i],
                                xt8_sb[:, tci, i],
                                start=first, stop=last, perf_mode=DR)
                        else:
                            nc.tensor.matmul(
                                box["ps"][:],
                                wqk_sb[:, i, wcol:wcol + P],
                                xt16_sb[:, tci, i],
                                start=first, stop=last)
                        if last:
                            if USE_FP8_KQ:
                                nc.vector.tensor_scalar(
                                    qk_sb[:, jcol, tsl], box["ps"][:],
                                    KQ_FIX, bqk_sb[:, jcol:jcol + 1],
                                    op0=mybir.AluOpType.mult,
                                    op1=mybir.AluOpType.add)
                            else:
                                nc.vector.tensor_scalar(
                                    qk_sb[:, jcol, tsl], box["ps"][:],
                                    bqk_sb[:, jcol:jcol + 1], None,
                                    op0=mybir.AluOpType.add)
                    return step
                for i in range(nsteps):
                    steps.append(mk(i))
                return steps

            def v_group(tglob):
                tci, tt = tglob // 4, tglob % 4
                steps = []
                box = {}

                def mk(k):
                    first, last = k == 0, k == KT - 1

                    def step():
                        if first:
                            box["ps"] = psf.tile([P, 512], F32, tag="psf",
                                                 name=f"v_{tglob}")
                        nc.tensor.matmul(
                            box["ps"][:],
                            xt16_sb[:, tci, k, tt * P:(tt + 1) * P],
                            wv_sb[:, k, :],
                            start=first, stop=last)
                        if last:
                            nc.vector.tensor_tensor(
                                vaug[:, tglob, :, 0:HD],
                                box["ps"][:].rearrange(
                                    "p (h d) -> p h d", h=HPC),
                                bvb_sb[:].rearrange("p (h d) -> p h d", h=HPC),
                                op=mybir.AluOpType.add)
                    return step
                for k in range(KT):
                    steps.append(mk(k))
                return steps

            ost_box = {}

            def outproj_part_group(tt, cc):
                """jt 0..2 partial accumulation (needs head pairs 0..2)."""
                steps = []
                box = {}

                def mk(jt):
                    first, last = jt == 0, jt == 2

                    def step():
                        if first:
                            box["ps"] = psf.tile([P, 512], F32, tag="psf",
                                                 name=f"opp_{tt}_{cc}")
                        nc.tensor.matmul(
                            box["ps"][:],
                            ot_sb[:, jt, tt * P:(tt + 1) * P],
                            wo_sb[:, jt, cc * 512:(cc + 1) * 512],
                            start=first, stop=last)
                        if last:
                            nc.vector.tensor_copy(
                                ost_part[:, tt, cc * 512:(cc + 1) * 512],
                                box["ps"][:])
                    return step
                for jt in range(3):
                    steps.append(mk(jt))
                return steps

            def outproj_final_group(tt, cc, drain=False):
                """jt 3 matmul (heads 6,7) + add of the jt0-2 partial.

                In the drain, the partial is instead pre-loaded into the
                psum through an identity matmul and the sum is evacuated by
                the (idle) scalar engine, keeping the tail off the DVE."""
                steps = []

                def step():
                    ps = psf.tile([P, 512], F32, tag="psf",
                                  name=f"opf_{tt}_{cc}")
                    if cc == 0:
                        ost_box[tt] = ostpool.tile(
                            [P, D], BF16, tag="ost", name=f"ost_{tt}")
                    if drain:
                        nc.tensor.matmul(
                            ps[:], ident_sb[:],
                            ost_part[:, tt, cc * 512:(cc + 1) * 512],
                            start=True, stop=False)
                    nc.tensor.matmul(
                        ps[:],
                        ot_sb[:, 3, tt * P:(tt + 1) * P],
                        wo_sb[:, 3, cc * 512:(cc + 1) * 512],
                        start=not drain, stop=True)
                    if drain:
                        nc.scalar.activation(
                            ost_box[tt][:, cc * 512:(cc + 1) * 512],
                            ps[:], COPY)
                    else:
                        nc.vector.tensor_tensor(
                            ost_box[tt][:, cc * 512:(cc + 1) * 512],
                            ps[:],
                            ost_part[:, tt, cc * 512:(cc + 1) * 512],
                            op=mybir.AluOpType.add)
                    if cc == 1:
                        eng = nc.sync if tt % 2 == 0 else nc.gpsimd
                        eng.dma_start(out_d[tt * P:(tt + 1) * P, :],
                                      ost_box[tt][:])
                steps.append(step)
                return steps

            # ---------------- preamble ----------------
            # Minimal work to unblock scores (0,0,0..3) and AV (0,0,0..1):
            # kT j0 first chunk, q j0 chunk 0, v tiles 0-1. The rest of the
            # j0 projections and v tiles are front-loaded filler.
            for s in kq_group(4, 0):
                s()
            for s in kq_group(0, 0):
                s()
            for s in v_group(0):
                s()
            for s in v_group(1):
                s()

            # ---------------- filler window assignments ----------------
            # win key = (j, c); each window spans 16 attention iters.
            win_steps = {}
            win_start = {}
            win_len = {}

            def assign(windows, groups, start=0, length=16):
                flat = [s for g in groups for s in g]
                n = len(windows)
                for i, w in enumerate(windows):
                    win_steps[w] = flat[len(flat) * i // n:
                                        len(flat) * (i + 1) // n]
                    win_start[w] = start
                    win_len[w] = length

            # (0,0): the rest of the j0/v prerequisites, ordered by due
            # date: kT j0 tiles 4-15 due at iters 3/7/11, v tile m due at
            # iter m+1+AV_LAG, q j0 chunk 1 due at iter 14.
            assign([(0, 0)],
                   [kq_group(4, 1), v_group(2), kq_group(4, 2), v_group(3),
                    v_group(4), kq_group(4, 3), v_group(5), v_group(6),
                    v_group(7), v_group(8), v_group(9), kq_group(0, 1),
                    v_group(10), v_group(11), v_group(12), v_group(13),
                    v_group(14), v_group(15)],
                   length=15)
            # remaining q j0 chunks + pair-1..3 projections, each a window
            # ahead of first use
            assign([(0, 1)], [kq_group(0, 2), kq_group(0, 3),
                              kq_group(5, 0), kq_group(5, 1)])
            assign([(0, 2)], [kq_group(5, 2), kq_group(5, 3),
                              kq_group(1, 0), kq_group(1, 1)])
            assign([(0, 3)], [kq_group(1, 2), kq_group(1, 3),
                              kq_group(6, 0), kq_group(6, 1)])
            assign([(1, 0)], [kq_group(6, 2), kq_group(6, 3),
                              kq_group(2, 0), kq_group(2, 1)])
            assign([(1, 1)], [kq_group(2, 2), kq_group(2, 3),
                              kq_group(7, 0), kq_group(7, 1)])
            assign([(1, 2)], [kq_group(7, 2), kq_group(7, 3),
                              kq_group(3, 0), kq_group(3, 1)])
            assign([(1, 3)], [kq_group(3, 2), kq_group(3, 3)])
            # out-projection partials (jt 0-2): chunk c's t-tiles are ready
            # once pairs 0-2 have evac'd+normalized chunk c; with the AV
            # lag, evac of unit u lands at iter 16u+16+AV_LAG.
            OPS = AV_LAG + 1
            assign([(2, 1)], [outproj_part_group(tt, cc)
                              for tt in range(0, 4) for cc in range(2)],
                   start=OPS, length=16 - OPS)
            assign([(2, 2)], [outproj_part_group(tt, cc)
                              for tt in range(4, 8) for cc in range(2)],
                   start=OPS, length=16 - OPS)
            assign([(2, 3)], [outproj_part_group(tt, cc)
                              for tt in range(8, 12) for cc in range(2)],
                   start=OPS, length=16 - OPS)
            assign([(3, 0)], [outproj_part_group(tt, cc)
                              for tt in range(12, 16) for cc in range(2)],
                   start=OPS, length=16 - OPS)
            # finals (jt 3 = pair 3): chunk c normalized at iter
            # 16*(12+c)+16+AV_LAG; give the norm DVE/DMA an extra head
            # start.
            FS = AV_LAG + 2
            assign([(3, 1)],
                   [outproj_final_group(tt, cc) for tt in range(0, 4)
                    for cc in range(2)], start=FS, length=16 - FS)
            assign([(3, 2)],
                   [outproj_final_group(tt, cc) for tt in range(4, 8)
                    for cc in range(2)], start=FS, length=16 - FS)
            assign([(3, 3)],
                   [outproj_final_group(tt, cc) for tt in range(8, 12)
                    for cc in range(2)], start=FS, length=16 - FS)

            # ---------------- attention stream ----------------
            # Per iteration (pair j, chunk c, k-tile): two concurrent
            # row-tiled K=64 score matmuls (head 2j in array rows 0-63,
            # head 2j+1 in rows 64-127) -> one [128,1024] exp -> two AV
            # matmuls accumulating per-head [o|denominator] psums.
            # Software pipeline: scores run one iteration ahead of exp;
            # AV trails by one iteration.
            iters = [(j, c, k)
                     for j in range(NPAIR) for c in range(TCH)
                     for k in range(TT)]
            pss_tiles = {}
            av_tiles = {}
            wt_tiles = {}

            def emit_scores(j, c, k):
                ps = pss.tile([P, 2, 512], F32, tag="pss",
                              name=f"s_{j}_{c}_{k}")
                kT2 = qk_sb[:, 4 + j, :]
                q2 = qk_sb[:, j, :]
                ksl = slice(k * P, (k + 1) * P)
                csl = slice(c * 512, (c + 1) * 512)
                nc.tensor.matmul(ps[:, 0, :], kT2[0:HD, ksl],
                                 q2[0:HD, csl], start=True, stop=True)
                nc.tensor.matmul(ps[:, 1, :], kT2[HD:P, ksl],
                                 q2[HD:P, csl], start=True, stop=True)
                pss_tiles[(j, c, k)] = ps

            def emit_av(pj, pc, pk):
                if pk == 0:
                    av_tiles[(pj, pc)] = avp.tile(
                        [P, 2, 512], F32, tag="av", name=f"av_{pj}_{pc}")
                pav = av_tiles[(pj, pc)]
                pwt = wt_tiles.pop((pj, pc, pk))
                first, last = pk == 0, pk == TT - 1
                # o: col-tiled concurrent pair -> bank 0, A rows 0-63,
                # B rows 64-127
                for a in range(2):
                    nc.tensor.matmul(
                        pav[a * HD:(a + 1) * HD, 0, :],
                        vaug_f[:, pk,
                               (2 * pj + a) * HD:(2 * pj + a + 1) * HD],
                        pwt[:, a * 512:(a + 1) * 512],
                        start=first, stop=last)
                # denominators: concurrent M=1 ones-matmuls -> bank 1,
                # A at partition 0, B at partition 32
                for a in range(2):
                    nc.tensor.matmul(
                        pav[32 * a:32 * a + 1, 1, :],
                        ones_av[:],
                        pwt[:, a * 512:(a + 1) * 512],
                        start=first, stop=last)

            def emit_evac_finish(pj, pc):
                pav = av_tiles.pop((pj, pc))
                csl = slice(pc * 512, (pc + 1) * 512)
                # o rows (both heads, partition-aligned) + denominator rows
                nc.vector.tensor_copy(ot_sb[:, pj, csl], pav[:, 0, :])
                dd = ddpool.tile([1, 1024], F32, tag="dd",
                                 name=f"dd_{pj}_{pc}")
                nc.vector.tensor_copy(dd[0:1, 0:512], pav[0:1, 1, :])
                nc.vector.tensor_copy(dd[0:1, 512:1024], pav[32:33, 1, :])
                rc = rcpool.tile([1, 1024], F32, tag="rc",
                                 name=f"rc_{pj}_{pc}")
                nc.vector.reciprocal_approx_fast(rc[0:1, :], dd[0:1, :])
                if pj == NPAIR - 1 and pc >= TCH - 2:
                    # tail: broadcast 1/den through the PE (fp32 rank-1
                    # matmuls) to skip the DRAM-bounce latency
                    for a in range(2):
                        rbp = psf.tile([P, 512], F32, tag="psf",
                                       name=f"rbp_{pc}_{a}")
                        nc.tensor.matmul(
                            rbp[a * HD:(a + 1) * HD, :],
                            ones1[0:1, :],
                            rc[0:1, a * 512:(a + 1) * 512],
                            start=True, stop=True)
                        nc.vector.tensor_tensor(
                            ot_sb[a * HD:(a + 1) * HD, pj, csl],
                            ot_sb[a * HD:(a + 1) * HD, pj, csl],
                            rbp[a * HD:(a + 1) * HD, :],
                            op=mybir.AluOpType.mult)
                else:
                    rd = rdpool.tile([2, 512], F32, tag="rd",
                                     name=f"rd_{pj}_{pc}")
                    nc.sync.dma_start(
                        rd[:].rearrange("a b -> (a b)"), rc[0:1, :])
                    rb = rbpool.tile([P, 512], F32, tag="rb",
                                     name=f"rb_{pj}_{pc}")
                    for a in range(2):
                        rd_bcast = bass.AP(
                            tensor=rd.tensor, offset=rd.offset + a * 512,
                            ap=[[0, HD], [1, 512]])
                        nc.sync.dma_start(rb[a * HD:(a + 1) * HD, :],
                                          rd_bcast)
                    nc.vector.tensor_mul(
                        ot_sb[:, pj, csl], ot_sb[:, pj, csl], rb[:, :])

            win_emitted = {w: 0 for w in win_steps}
            emit_scores(*iters[0])
            for gi, (j, c, k) in enumerate(iters):
                if gi + 1 < len(iters):
                    emit_scores(*iters[gi + 1])
                ai = gi - 1 - AV_LAG
                av_it = iters[ai] if ai >= 0 else None
                # unit-boundary AV + evac go first so the DVE evacuation
                # frees the AV psum before the next unit's k=0 AV
                if av_it is not None and av_it[2] == TT - 1:
                    emit_av(*av_it)
                    emit_evac_finish(av_it[0], av_it[1])
                w = (j, c)
                steps = win_steps.get(w, [])
                if steps:
                    s0 = win_start.get(w, 0)
                    slen = win_len.get(w, 16)
                    if k >= s0:
                        tgt = len(steps) if k == TT - 1 else min(
                            len(steps),
                            -(-len(steps) * (k - s0 + 1) // slen)
                            + (3 if k == s0 and w == (0, 0) else 0))
                        while win_emitted[w] < tgt:
                            steps[win_emitted[w]]()
                            win_emitted[w] += 1
                wt = wtpool.tile([P, 1024], BF16, tag="wt",
                                 name=f"wt_{j}_{c}_{k}")
                nc.scalar.activation(
                    wt[:], pss_tiles.pop((j, c, k))[:].rearrange(
                        "p a b -> p (a b)"),
                    EXP, bias=0.0, scale=SC)
                wt_tiles[(j, c, k)] = wt
                if av_it is not None and av_it[2] != TT - 1:
                    emit_av(*av_it)

            # drain: lagged AVs + finish + outproj finals for tiles 12..15
            for ai in range(len(iters) - 1 - AV_LAG, len(iters)):
                av_it = iters[ai]
                emit_av(*av_it)
                if av_it[2] == TT - 1:
                    emit_evac_finish(av_it[0], av_it[1])
            for tt in range(12, 16):
                for cc in range(2):
                    for s in outproj_final_group(tt, cc, drain=True):
                        s()

    nc.compile()
    return nc


def _prep_core_inputs(x, qkv_w, qkv_b, out_w, core):
    b, g = core // 2, core % 2
    jsl = slice(g * JC, (g + 1) * JC)

    xT = np.ascontiguousarray(x[b].T)                       # [1024, 2048]
    xk = xT.reshape(KT, P, TCH, 512)
    xt16 = np.ascontiguousarray(xk.transpose(2, 1, 0, 3))   # [4,128,8,512]

    Wq = qkv_w[0 * D:1 * D][jsl]                            # [512, 1024]
    Wk = qkv_w[1 * D:2 * D][jsl]
    Wv = qkv_w[2 * D:3 * D][jsl]
    WqkT = np.concatenate([Wq, Wk], axis=0).T               # [1024, 1024]
    WvT = Wv.T                                              # [1024, 512]
    wv = np.ascontiguousarray(
        WvT.reshape(KT, P, JC).transpose(1, 0, 2))          # [128, 8, 512]

    bq = qkv_b[0 * D:1 * D][jsl]
    bk = qkv_b[1 * D:2 * D][jsl]
    bv = qkv_b[2 * D:3 * D][jsl]
    bqk = np.ascontiguousarray(
        np.concatenate([bq, bk]).reshape(8, P).T)           # [128, 8]
    bvb = np.ascontiguousarray(np.tile(bv[None, :], (P, 1)))

    WoT = np.ascontiguousarray(out_w[:, jsl].T)             # [512, 1024]
    wo = np.ascontiguousarray(
        WoT.reshape(JC // P, P, D).transpose(1, 0, 2))      # [128, 4, 1024]

    inputs = {
        "xt16": xt16.astype(BF16_NP),
        "ident": np.eye(P, dtype=BF16_NP),
        "wv": wv.astype(BF16_NP),
        "wo": wo.astype(BF16_NP),
        "bqk": bqk.astype(np.float32),
        "bvb": bvb.astype(BF16_NP),
    }
    if USE_FP8_KQ:
        xt8 = np.ascontiguousarray(
            (xk * X8_SCALE).reshape(KT // 2, 2, P, TCH, 512)
            .transpose(3, 2, 0, 1, 4))                      # [4,128,4,2,512]
        wqk8 = np.ascontiguousarray(
            (WqkT * W8_SCALE).reshape(KT // 2, 2, P, 8, P)
            .transpose(2, 3, 0, 1, 4))                      # [128,8,4,2,128]
        inputs["xt8"] = xt8.astype(FP8_NP)
        inputs["wqk"] = wqk8.astype(FP8_NP)
    else:
        wqk = np.ascontiguousarray(
            WqkT.reshape(KT, P, 2 * JC).transpose(1, 0, 2))  # [128, 8, 1024]
        inputs["wqk"] = wqk.astype(BF16_NP)
    return inputs


def run(x, qkv_w, qkv_b, out_w, out_b, trace=False, tmpdir=None):
    if "nc" not in _cached:
        _cached["nc"] = build_program()
    nc = _cached["nc"]
    in_maps = [_prep_core_inputs(x, qkv_w, qkv_b, out_w, c) for c in range(NC)]
    res = run_bass_kernel_spmd(nc, in_maps, core_ids=list(range(NC)),
                               trace=trace, tmpdir=tmpdir)
    parts = np.stack([np.asarray(res.results[c]["out"], dtype=np.float32)
                      for c in range(NC)])                  # [8, T, D]
    out = parts.reshape(B, 2, T, D).sum(axis=1) + out_b[None, None, :]
    return out.astype(np.float32), res


def kernel(x, qkv_w, qkv_b, out_w, out_b):
    x = np.asarray(x, dtype=np.float32)
    qkv_w = np.asarray(qkv_w, dtype=np.float32)
    qkv_b = np.asarray(qkv_b, dtype=np.float32)
    out_w = np.asarray(out_w, dtype=np.float32)
    out_b = np.asarray(out_b, dtype=np.float32)
    out, _ = run(x, qkv_w, qkv_b, out_w, out_b, trace=False)
    return out


if __name__ == "__main__":
    import jax
    import reference
    inputs = {k: np.asarray(v) for k, v in reference.setup_inputs().items()}
    expected = np.asarray(reference.reference(**inputs))
    got = kernel(**inputs)
    err = np.linalg.norm(got - expected) / np.linalg.norm(expected)
    print("Relative error:", err)


# revision 19
# speedup vs baseline: 1.1340x; 1.1340x over previous
"""Multi-head attention (B=4, T=2048, D=1024, H=16) on 8 Trainium2 cores.

Sharding: batch (4-way) x head-half (2-way) -> 8 cores.
Core c handles batch b = c//2 and heads g*8..g*8+8 where g = c%2.

v3: row-tiled concurrent score matmuls.
  - Scores for a HEAD PAIR run as two concurrent K=64 matmuls in the two
    64-row halves of the PE array (tile_position row tiling, auto-derived
    from the AP base partitions). qk_sb already stores each j-tile as
    [128 = headpair hd, T], so head 2j lives in partitions 0-63 and head
    2j+1 in 64-127 for both q and kT: no zero-padded qpad staging needed.
  - Loop iterates (pair j, query-chunk c of 512, k-tile). Each iteration:
    2 concurrent score matmuls -> one [128,1024] exp (chunk for A || B)
    -> 2 AV matmuls (one per head, ones-column augmented v for the
    denominators). PSUM: scores 2x2 banks (double buffer), AV 2 banks,
    filler 2 banks = 8.
  - Softmax denominator row copies moved off the scalar engine (DVE) so
    ACT runs pure Exp back-to-back; normalization per (pair, chunk) is
    one reciprocal + DRAM-bounce broadcast + one [128,512] DVE multiply
    for both heads at once. Last two units broadcast 1/d through the PE
    instead (rank-1 fp32 matmuls) to skip the bounce latency on the tail.
  - q/k projections run as fp8e4 DoubleRow matmuls (2 k-tiles per
    instruction). Inputs are scaled (x*16, W*64) into the fp8 normal
    range; the 1/1024 fixup is folded into the psum->SBUF evacuation.
    v stays bf16. Biases fold into the psum evacuations.
  - Filler GEMMs (remaining projections + out-projection) weave into the
    attention stream per 16-iteration windows with due-date slopes.
  - Output is bf16 (halves DMA); host sums the two head-half partials.

Host: transposes/reshapes inputs per core (bf16/fp8), sums partials,
adds out_b.
"""

import numpy as np
import ml_dtypes
from contextlib import ExitStack

import concourse.bass as bass
import concourse.tile as tile
from concourse import bacc, mybir
from concourse.bass_utils import run_bass_kernel_spmd

BF16_NP = ml_dtypes.bfloat16
FP8_NP = ml_dtypes.float8_e4m3

B, T, D = 4, 2048, 1024
H, HD = 16, 64
P = 128
NC = 8
HPC = 8          # heads per core
JC = HPC * HD    # 512 head-dim columns per core
KT = D // P      # 8 contraction tiles for QKV
TT = T // P      # 16 t tiles
TCH = T // 512   # 4 t chunks of 512
NPAIR = HPC // 2  # 4 head pairs per core
F32 = mybir.dt.float32
BF16 = mybir.dt.bfloat16
FP8 = mybir.dt.float8e4

USE_FP8_KQ = True
X8_SCALE = 16.0
W8_SCALE = 64.0
KQ_FIX = 1.0 / (X8_SCALE * W8_SCALE)

_cached = {}


def build_program():
    nc = bacc.Bacc("TRN2", target_bir_lowering=False, debug=False,
                   enable_asserts=True, num_devices=NC)

    xt16_d = nc.dram_tensor("xt16", [TCH, P, KT, 512], BF16,
                            kind="ExternalInput").ap()
    if USE_FP8_KQ:
        xt8_d = nc.dram_tensor("xt8", [TCH, P, KT // 2, 2, 512], FP8,
                               kind="ExternalInput").ap()
        wqk_d = nc.dram_tensor("wqk", [P, 8, KT // 2, 2, P], FP8,
                               kind="ExternalInput").ap()
    else:
        wqk_d = nc.dram_tensor("wqk", [P, KT, 2 * JC], BF16,
                               kind="ExternalInput").ap()
    wv_d = nc.dram_tensor("wv", [P, KT, JC], BF16, kind="ExternalInput").ap()
    wo_d = nc.dram_tensor("wo", [P, JC // P, D], BF16,
                          kind="ExternalInput").ap()
    bqk_d = nc.dram_tensor("bqk", [P, 8], F32, kind="ExternalInput").ap()
    ident_d = nc.dram_tensor("ident", [P, P], BF16, kind="ExternalInput").ap()
    bvb_d = nc.dram_tensor("bvb", [P, JC], BF16, kind="ExternalInput").ap()
    out_d = nc.dram_tensor("out", [T, D], BF16, kind="ExternalOutput").ap()

    EXP = mybir.ActivationFunctionType.Exp
    COPY = mybir.ActivationFunctionType.Copy
    DR = mybir.MatmulPerfMode.DoubleRow
    SC = 0.125  # 1/sqrt(HD)

    with tile.TileContext(nc) as tc:
        with ExitStack() as ctx:
            persist = ctx.enter_context(tc.tile_pool(name="persist", bufs=1))
            xt16_sb = persist.tile([P, TCH, KT, 512], BF16, tag="xt16")
            if USE_FP8_KQ:
                xt8_sb = persist.tile([P, TCH, KT // 2, 2, 512], FP8,
                                      tag="xt8")
                wqk_sb = persist.tile([P, 8, KT // 2, 2, P], FP8, tag="wqk")
            else:
                wqk_sb = persist.tile([P, KT, 2 * JC], BF16, tag="wqk")
            wv_sb = persist.tile([P, KT, JC], BF16, tag="wv")
            wo_sb = persist.tile([P, JC // P, D], BF16, tag="wo")
            bqk_sb = persist.tile([P, 8], F32, tag="bqk")
            ident_sb = persist.tile([P, P], BF16, tag="ident")
            bvb_sb = persist.tile([P, JC], BF16, tag="bvb")
            qk_sb = persist.tile([P, 8, T], BF16, tag="qk")
            # [t, 8 x [v(64)|1]] + 64 pad cols so the AV stationary operand
            # can be sliced 128 wide
            VW = HPC * (HD + 1)
            vaug_f = persist.tile([P, TT, VW + HD], BF16, tag="vaug")
            ot_sb = persist.tile([P, JC // P, T], BF16, tag="ot")
            # out-projection partial sums over head-pairs 0..2 (bf16), so
            # most of the out-proj runs in the late windows
            ost_part = persist.tile([P, TT, D], BF16, tag="ostp")

            # ---- input DMAs (program order = queue order) ----
            # Critical path first: wqk j-tiles 4 and 0 + xt8 chunk 0 unblock
            # the first score/exp; wqk is split per j-tile so the first
            # pieces don't wait on the full 1MB transfer.
            if USE_FP8_KQ:
                for tci in range(2):
                    nc.sync.dma_start(xt8_sb[:, tci], xt8_d[tci])
                nc.sync.dma_start(xt16_sb[:, 0], xt16_d[0])
                for tci in range(2, TCH):
                    nc.sync.dma_start(xt8_sb[:, tci], xt8_d[tci])
                for tci in range(1, TCH):
                    nc.sync.dma_start(xt16_sb[:, tci], xt16_d[tci])
                for jc in (4, 0):
                    nc.gpsimd.dma_start(wqk_sb[:, jc], wqk_d[:, jc])
            else:
                for tci in range(TCH):
                    nc.sync.dma_start(xt16_sb[:, tci], xt16_d[tci])
                nc.gpsimd.dma_start(wqk_sb[:], wqk_d[:])
            nc.gpsimd.dma_start(bqk_sb[:], bqk_d[:])
            nc.gpsimd.dma_start(wv_sb[:], wv_d[:])
            nc.gpsimd.dma_start(bvb_sb[:], bvb_d[:])
            if USE_FP8_KQ:
                for jc in (5, 1, 6, 2, 7, 3):
                    nc.gpsimd.dma_start(wqk_sb[:, jc], wqk_d[:, jc])
            nc.gpsimd.dma_start(ident_sb[:], ident_d[:])
            nc.gpsimd.dma_start(wo_sb[:], wo_d[:])

            ones1 = persist.tile([1, HD], F32, tag="ones1")
            nc.gpsimd.memset(ones1[:], 1.0)
            vaug = vaug_f[:, :, 0:VW].rearrange(
                "p t (h e) -> p t h e", h=HPC)          # [128, 16, 8, 65]
            for tt in range(TT):
                nc.gpsimd.memset(vaug[:, tt, :, HD:HD + 1], 1.0)
                nc.gpsimd.memset(vaug_f[:, tt, VW:VW + HD], 0.0)

            AV_LAG = 4  # AV trails exp by this many extra iterations
            wtpool = ctx.enter_context(
                tc.tile_pool(name="wtpool", bufs=AV_LAG + 3))
            ddpool = ctx.enter_context(tc.tile_pool(name="ddpool", bufs=1))
            rcpool = ctx.enter_context(tc.tile_pool(name="rcpool", bufs=2))
            rbpool = ctx.enter_context(tc.tile_pool(name="rbpool", bufs=2))
            ostpool = ctx.enter_context(tc.tile_pool(name="ostpool", bufs=2))
            rdpool = ctx.enter_context(
                tc.tile_pool(name="rdpool", bufs=2, space="DRAM"))
            pss = ctx.enter_context(
                tc.tile_pool(name="pss", bufs=2, space="PSUM"))
            avp = ctx.enter_context(
                tc.tile_pool(name="avp", bufs=1, space="PSUM"))
            psf = ctx.enter_context(
                tc.tile_pool(name="psf", bufs=2, space="PSUM"))

            # ---------------- filler group builders ----------------
            # Each group is a list of closures; each closure emits one PE
            # matmul (the last also emits the psum evacuation on DVE).

            def kq_group(jcol, tci):
                """qk_sb j-tile jcol (0-3 = q j, 4-7 = kT j) over t-chunk."""
                wcol = jcol * P if jcol < 4 else JC + (jcol - 4) * P
                tsl = slice(tci * 512, (tci + 1) * 512)
                steps = []
                box = {}
                nsteps = KT // 2 if USE_FP8_KQ else KT

                def mk(i):
                    first, last = i == 0, i == nsteps - 1

                    def step():
                        if first:
                            box["ps"] = psf.tile([P, 512], F32, tag="psf",
                                                 name=f"kq_{jcol}_{tci}")
                        if USE_FP8_KQ:
                            nc.tensor.matmul(
                                box["ps"][:],
                                wqk_sb[:, jcol, i],
                                xt8_sb[:, tci, i],
                                start=first, stop=last, perf_mode=DR)
                        else:
                            nc.tensor.matmul(
                                box["ps"][:],
                                wqk_sb[:, i, wcol:wcol + P],
                                xt16_sb[:, tci, i],
                                start=first, stop=last)
                        if last:
                            if USE_FP8_KQ:
                                nc.vector.tensor_scalar(
                                    qk_sb[:, jcol, tsl], box["ps"][:],
                                    KQ_FIX, bqk_sb[:, jcol:jcol + 1],
                                    op0=mybir.AluOpType.mult,
                                    op1=mybir.AluOpType.add)
                            else:
                                nc.vector.tensor_scalar(
                                    qk_sb[:, jcol, tsl], box["ps"][:],
                                    bqk_sb[:, jcol:jcol + 1], None,
                                    op0=mybir.AluOpType.add)
                    return step
                for i in range(nsteps):
                    steps.append(mk(i))
                return steps

            def v_group(tglob):
                tci, tt = tglob // 4, tglob % 4
                steps = []
                box = {}

                def mk(k):
                    first, last = k == 0, k == KT - 1

                    def step():
                        if first:
                            box["ps"] = psf.tile([P, 512], F32, tag="psf",
                                                 name=f"v_{tglob}")
                        nc.tensor.matmul(
                            box["ps"][:],
                            xt16_sb[:, tci, k, tt * P:(tt + 1) * P],
                            wv_sb[:, k, :],
                            start=first, stop=last)
                        if last:
                            nc.vector.tensor_tensor(
                                vaug[:, tglob, :, 0:HD],
                                box["ps"][:].rearrange(
                                    "p (h d) -> p h d", h=HPC),
                                bvb_sb[:].rearrange("p (h d) -> p h d", h=HPC),
                                op=mybir.AluOpType.add)
                    return step
                for k in range(KT):
                    steps.append(mk(k))
                return steps

            ost_box = {}

            def outproj_part_group(tt, cc):
                """jt 0..2 partial accumulation (needs head pairs 0..2)."""
                steps = []
                box = {}

                def mk(jt):
                    first, last = jt == 0, jt == 2

                    def step():
                        if first:
                            box["ps"] = psf.tile([P, 512], F32, tag="psf",
                                                 name=f"opp_{tt}_{cc}")
                        nc.tensor.matmul(
                            box["ps"][:],
                            ot_sb[:, jt, tt * P:(tt + 1) * P],
                            wo_sb[:, jt, cc * 512:(cc + 1) * 512],
                            start=first, stop=last)
                        if last:
                            nc.vector.tensor_copy(
                                ost_part[:, tt, cc * 512:(cc + 1) * 512],
                                box["ps"][:])
                    return step
                for jt in range(3):
                    steps.append(mk(jt))
                return steps

            def outproj_final_group(tt, cc, drain=False):
                """jt 3 matmul (heads 6,7) + add of the jt0-2 partial.

                In the drain, the partial is instead pre-loaded into the
                psum through an identity matmul and the sum is evacuated by
                the (idle) scalar engine, keeping the tail off the DVE."""
                steps = []

                def step():
                    ps = psf.tile([P, 512], F32, tag="psf",
                                  name=f"opf_{tt}_{cc}")
                    if cc == 0:
                        ost_box[tt] = ostpool.tile(
                            [P, D], BF16, tag="ost", name=f"ost_{tt}")
                    if drain:
                        nc.tensor.matmul(
                            ps[:], ident_sb[:],
                            ost_part[:, tt, cc * 512:(cc + 1) * 512],
                            start=True, stop=False)
                    nc.tensor.matmul(
                        ps[:],
                        ot_sb[:, 3, tt * P:(tt + 1) * P],
                        wo_sb[:, 3, cc * 512:(cc + 1) * 512],
                        start=not drain, stop=True)
                    if drain:
                        nc.scalar.activation(
                            ost_box[tt][:, cc * 512:(cc + 1) * 512],
                            ps[:], COPY)
                    else:
                        nc.vector.tensor_tensor(
                            ost_box[tt][:, cc * 512:(cc + 1) * 512],
                            ps[:],
                            ost_part[:, tt, cc * 512:(cc + 1) * 512],
                            op=mybir.AluOpType.add)
                    if cc == 1:
                        eng = nc.sync if tt % 2 == 0 else nc.gpsimd
                        eng.dma_start(out_d[tt * P:(tt + 1) * P, :],
                                      ost_box[tt][:])
                steps.append(step)
                return steps

            # ---------------- preamble ----------------
            # Minimal work to unblock scores (0,0,0..3): kT j0 first chunk
            # and q j0 chunk 0. Everything else (v tiles, the rest of the
            # j0 projections) is front-loaded filler; the AV lag gives the
            # v tiles until iter k+1+AV_LAG.
            for s in kq_group(4, 0):
                s()
            for s in kq_group(0, 0):
                s()

            # ---------------- filler window assignments ----------------
            # win key = (j, c); each window spans 16 attention iters.
            win_steps = {}
            win_start = {}
            win_len = {}

            def assign(windows, groups, start=0, length=16):
                flat = [s for g in groups for s in g]
                n = len(windows)
                for i, w in enumerate(windows):
                    win_steps[w] = flat[len(flat) * i // n:
                                        len(flat) * (i + 1) // n]
                    win_start[w] = start
                    win_len[w] = length

            # (0,0): the rest of the j0/v prerequisites, ordered by due
            # date: kT j0 tiles 4-15 due at iters 3/7/11, v tile m due at
            # iter m+1+AV_LAG, q j0 chunk 1 due at iter 14.
            assign([(0, 0)],
                   [kq_group(4, 1), v_group(0), v_group(1), v_group(2),
                    kq_group(4, 2), v_group(3), v_group(4), kq_group(4, 3),
                    v_group(5), v_group(6), v_group(7), v_group(8),
                    v_group(9), kq_group(0, 1), v_group(10), v_group(11),
                    v_group(12), v_group(13), v_group(14), v_group(15)],
                   length=15)
            # remaining q j0 chunks + pair-1..3 projections, each a window
            # ahead of first use
            assign([(0, 1)], [kq_group(0, 2), kq_group(0, 3),
                              kq_group(5, 0), kq_group(5, 1)])
            assign([(0, 2)], [kq_group(5, 2), kq_group(5, 3),
                              kq_group(1, 0), kq_group(1, 1)])
            assign([(0, 3)], [kq_group(1, 2), kq_group(1, 3),
                              kq_group(6, 0), kq_group(6, 1)])
            assign([(1, 0)], [kq_group(6, 2), kq_group(6, 3),
                              kq_group(2, 0), kq_group(2, 1)])
            assign([(1, 1)], [kq_group(2, 2), kq_group(2, 3),
                              kq_group(7, 0), kq_group(7, 1)])
            assign([(1, 2)], [kq_group(7, 2), kq_group(7, 3),
                              kq_group(3, 0), kq_group(3, 1)])
            assign([(1, 3)], [kq_group(3, 2), kq_group(3, 3)])
            # out-projection partials (jt 0-2): chunk c's t-tiles are ready
            # once pairs 0-2 have evac'd+normalized chunk c; with the AV
            # lag, evac of unit u lands at iter 16u+16+AV_LAG.
            OPS = AV_LAG + 1
            assign([(2, 1)], [outproj_part_group(tt, cc)
                              for tt in range(0, 4) for cc in range(2)],
                   start=OPS, length=16 - OPS)
            assign([(2, 2)], [outproj_part_group(tt, cc)
                              for tt in range(4, 8) for cc in range(2)],
                   start=OPS, length=16 - OPS)
            assign([(2, 3)], [outproj_part_group(tt, cc)
                              for tt in range(8, 12) for cc in range(2)],
                   start=OPS, length=16 - OPS)
            assign([(3, 0)], [outproj_part_group(tt, cc)
                              for tt in range(12, 16) for cc in range(2)],
                   start=OPS, length=16 - OPS)
            # finals (jt 3 = pair 3): chunk c normalized at iter
            # 16*(12+c)+16+AV_LAG; give the norm DVE/DMA an extra head
            # start.
            FS = AV_LAG + 2
            assign([(3, 1)],
                   [outproj_final_group(tt, cc) for tt in range(0, 4)
                    for cc in range(2)], start=FS, length=16 - FS)
            assign([(3, 2)],
                   [outproj_final_group(tt, cc) for tt in range(4, 8)
                    for cc in range(2)], start=FS, length=16 - FS)
            assign([(3, 3)],
                   [outproj_final_group(tt, cc) for tt in range(8, 12)
                    for cc in range(2)], start=FS, length=16 - FS)

            # ---------------- attention stream ----------------
            # Per iteration (pair j, chunk c, k-tile): two concurrent
            # row-tiled K=64 score matmuls (head 2j in array rows 0-63,
            # head 2j+1 in rows 64-127) -> one [128,1024] exp -> two AV
            # matmuls accumulating per-head [o|denominator] psums.
            # Software pipeline: scores run one iteration ahead of exp;
            # AV trails by one iteration.
            iters = [(j, c, k)
                     for j in range(NPAIR) for c in range(TCH)
                     for k in range(TT)]
            pss_tiles = {}
            av_tiles = {}
            wt_tiles = {}

            def emit_scores(j, c, k):
                ps = pss.tile([P, 2, 512], F32, tag="pss",
                              name=f"s_{j}_{c}_{k}")
                kT2 = qk_sb[:, 4 + j, :]
                q2 = qk_sb[:, j, :]
                ksl = slice(k * P, (k + 1) * P)
                csl = slice(c * 512, (c + 1) * 512)
                nc.tensor.matmul(ps[:, 0, :], kT2[0:HD, ksl],
                                 q2[0:HD, csl], start=True, stop=True)
                nc.tensor.matmul(ps[:, 1, :], kT2[HD:P, ksl],
                                 q2[HD:P, csl], start=True, stop=True)
                pss_tiles[(j, c, k)] = ps

            def emit_av(pj, pc, pk):
                if pk == 0:
                    av_tiles[(pj, pc)] = avp.tile(
                        [P, 2, 512], F32, tag="av", name=f"av_{pj}_{pc}")
                pav = av_tiles[(pj, pc)]
                pwt = wt_tiles.pop((pj, pc, pk))
                for a in range(2):
                    nc.tensor.matmul(
                        pav[:, a, :],
                        vaug_f[:, pk, (2 * pj + a) * (HD + 1):
                               (2 * pj + a) * (HD + 1) + P],
                        pwt[:, a * 512:(a + 1) * 512],
                        start=(pk == 0), stop=(pk == TT - 1))

            def emit_evac_finish(pj, pc):
                pav = av_tiles.pop((pj, pc))
                csl = slice(pc * 512, (pc + 1) * 512)
                # o rows (both heads) + denominator rows, all on DVE
                nc.vector.tensor_copy(ot_sb[0:HD, pj, csl], pav[0:HD, 0, :])
                nc.vector.tensor_copy(ot_sb[HD:P, pj, csl], pav[0:HD, 1, :])
                dd = ddpool.tile([1, 1024], F32, tag="dd",
                                 name=f"dd_{pj}_{pc}")
                nc.vector.tensor_copy(dd[0:1, 0:512], pav[HD:HD + 1, 0, :])
                nc.vector.tensor_copy(dd[0:1, 512:1024],
                                      pav[HD:HD + 1, 1, :])
                rc = rcpool.tile([1, 1024], F32, tag="rc",
                                 name=f"rc_{pj}_{pc}")
                nc.vector.reciprocal_approx_fast(rc[0:1, :], dd[0:1, :])
                if pj == NPAIR - 1 and pc >= TCH - 2:
                    # tail: broadcast 1/den through the PE (fp32 rank-1
                    # matmuls) to skip the DRAM-bounce latency
                    for a in range(2):
                        rbp = psf.tile([P, 512], F32, tag="psf",
                                       name=f"rbp_{pc}_{a}")
                        nc.tensor.matmul(
                            rbp[a * HD:(a + 1) * HD, :],
                            ones1[0:1, :],
                            rc[0:1, a * 512:(a + 1) * 512],
                            start=True, stop=True)
                        nc.vector.tensor_tensor(
                            ot_sb[a * HD:(a + 1) * HD, pj, csl],
                            ot_sb[a * HD:(a + 1) * HD, pj, csl],
                            rbp[a * HD:(a + 1) * HD, :],
                            op=mybir.AluOpType.mult)
                else:
                    rd = rdpool.tile([2, 512], F32, tag="rd",
                                     name=f"rd_{pj}_{pc}")
                    nc.sync.dma_start(
                        rd[:].rearrange("a b -> (a b)"), rc[0:1, :])
                    rb = rbpool.tile([P, 512], F32, tag="rb",
                                     name=f"rb_{pj}_{pc}")
                    for a in range(2):
                        rd_bcast = bass.AP(
                            tensor=rd.tensor, offset=rd.offset + a * 512,
                            ap=[[0, HD], [1, 512]])
                        nc.sync.dma_start(rb[a * HD:(a + 1) * HD, :],
                                          rd_bcast)
                    nc.vector.tensor_mul(
                        ot_sb[:, pj, csl], ot_sb[:, pj, csl], rb[:, :])

            win_emitted = {w: 0 for w in win_steps}
            emit_scores(*iters[0])
            av_cursor = 0
            n_it = len(iters)

            def av_target(g):
                # lag tapers to 0 at the last iteration so the drain only
                # holds the final AV
                return g - 1 - min(AV_LAG, n_it - 1 - g)

            for gi, (j, c, k) in enumerate(iters):
                if gi + 1 < n_it:
                    emit_scores(*iters[gi + 1])
                # unit-boundary AV + evac go first so the DVE evacuation
                # frees the AV psum before the next unit's k=0 AV
                while (av_cursor <= av_target(gi)
                       and iters[av_cursor][2] == TT - 1):
                    it = iters[av_cursor]
                    emit_av(*it)
                    emit_evac_finish(it[0], it[1])
                    av_cursor += 1
                w = (j, c)
                steps = win_steps.get(w, [])
                if steps:
                    s0 = win_start.get(w, 0)
                    slen = win_len.get(w, 16)
                    if k >= s0:
                        tgt = len(steps) if k == TT - 1 else min(
                            len(steps),
                            -(-len(steps) * (k - s0 + 1) // slen)
                            + (3 if k == s0 and w == (0, 0) else 0))
                        while win_emitted[w] < tgt:
                            steps[win_emitted[w]]()
                            win_emitted[w] += 1
                wt = wtpool.tile([P, 1024], BF16, tag="wt",
                                 name=f"wt_{j}_{c}_{k}")
                nc.scalar.activation(
                    wt[:], pss_tiles.pop((j, c, k))[:].rearrange(
                        "p a b -> p (a b)"),
                    EXP, bias=0.0, scale=SC)
                wt_tiles[(j, c, k)] = wt
                while av_cursor <= av_target(gi):
                    it = iters[av_cursor]
                    emit_av(*it)
                    if it[2] == TT - 1:
                        emit_evac_finish(it[0], it[1])
                    av_cursor += 1

            # drain: final AV + finish + outproj finals for tiles 12..15
            while av_cursor < n_it:
                it = iters[av_cursor]
                emit_av(*it)
                if it[2] == TT - 1:
                    emit_evac_finish(it[0], it[1])
                av_cursor += 1
            for tt in range(12, 16):
                for cc in range(2):
                    for s in outproj_final_group(tt, cc, drain=True):
                        s()

    nc.compile()
    return nc


def _prep_core_inputs(x, qkv_w, qkv_b, out_w, core):
    b, g = core // 2, core % 2
    jsl = slice(g * JC, (g + 1) * JC)

    xT = np.ascontiguousarray(x[b].T)                       # [1024, 2048]
    xk = xT.reshape(KT, P, TCH, 512)
    xt16 = np.ascontiguousarray(xk.transpose(2, 1, 0, 3))   # [4,128,8,512]

    Wq = qkv_w[0 * D:1 * D][jsl]                            # [512, 1024]
    Wk = qkv_w[1 * D:2 * D][jsl]
    Wv = qkv_w[2 * D:3 * D][jsl]
    WqkT = np.concatenate([Wq, Wk], axis=0).T               # [1024, 1024]
    WvT = Wv.T                                              # [1024, 512]
    wv = np.ascontiguousarray(
        WvT.reshape(KT, P, JC).transpose(1, 0, 2))          # [128, 8, 512]

    bq = qkv_b[0 * D:1 * D][jsl]
    bk = qkv_b[1 * D:2 * D][jsl]
    bv = qkv_b[2 * D:3 * D][jsl]
    bqk = np.ascontiguousarray(
        np.concatenate([bq, bk]).reshape(8, P).T)           # [128, 8]
    bvb = np.ascontiguousarray(np.tile(bv[None, :], (P, 1)))

    WoT = np.ascontiguousarray(out_w[:, jsl].T)             # [512, 1024]
    wo = np.ascontiguousarray(
        WoT.reshape(JC // P, P, D).transpose(1, 0, 2))      # [128, 4, 1024]

    inputs = {
        "xt16": xt16.astype(BF16_NP),
        "ident": np.eye(P, dtype=BF16_NP),
        "wv": wv.astype(BF16_NP),
        "wo": wo.astype(BF16_NP),
        "bqk": bqk.astype(np.float32),
        "bvb": bvb.astype(BF16_NP),
    }
    if USE_FP8_KQ:
        xt8 = np.ascontiguousarray(
            (xk * X8_SCALE).reshape(KT // 2, 2, P, TCH, 512)
            .transpose(3, 2, 0, 1, 4))                      # [4,128,4,2,512]
        wqk8 = np.ascontiguousarray(
            (WqkT * W8_SCALE).reshape(KT // 2, 2, P, 8, P)
            .transpose(2, 3, 0, 1, 4))                      # [128,8,4,2,128]
        inputs["xt8"] = xt8.astype(FP8_NP)
        inputs["wqk"] = wqk8.astype(FP8_NP)
    else:
        wqk = np.ascontiguousarray(
            WqkT.reshape(KT, P, 2 * JC).transpose(1, 0, 2))  # [128, 8, 1024]
        inputs["wqk"] = wqk.astype(BF16_NP)
    return inputs


def run(x, qkv_w, qkv_b, out_w, out_b, trace=False, tmpdir=None):
    if "nc" not in _cached:
        _cached["nc"] = build_program()
    nc = _cached["nc"]
    in_maps = [_prep_core_inputs(x, qkv_w, qkv_b, out_w, c) for c in range(NC)]
    res = run_bass_kernel_spmd(nc, in_maps, core_ids=list(range(NC)),
                               trace=trace, tmpdir=tmpdir)
    parts = np.stack([np.asarray(res.results[c]["out"], dtype=np.float32)
                      for c in range(NC)])                  # [8, T, D]
    out = parts.reshape(B, 2, T, D).sum(axis=1) + out_b[None, None, :]
    return out.astype(np.float32), res


def kernel(x, qkv_w, qkv_b, out_w, out_b):
    x = np.asarray(x, dtype=np.float32)
    qkv_w = np.asarray(qkv_w, dtype=np.float32)
    qkv_b = np.asarray(qkv_b, dtype=np.float32)
    out_w = np.asarray(out_w, dtype=np.float32)
    out_b = np.asarray(out_b, dtype=np.float32)
    out, _ = run(x, qkv_w, qkv_b, out_w, out_b, trace=False)
    return out


if __name__ == "__main__":
    import jax
    import reference
    inputs = {k: np.asarray(v) for k, v in reference.setup_inputs().items()}
    expected = np.asarray(reference.reference(**inputs))
    got = kernel(**inputs)
    err = np.linalg.norm(got - expected) / np.linalg.norm(expected)
    print("Relative error:", err)
